# revision 5
# baseline (speedup 1.0000x reference)
"""Cached multi-head attention (decode-append, S=4) on 8 Trainium2 NeuronCores.

Sharding: tensor-parallel over the 32 heads -> 4 heads per core.
  - Wq/Wk/Wv split on the output-feature (head) axis, Wo on the input axis.
  - Each core holds its heads' slice of the KV cache (positions 0..4095; the
    4 new positions are computed on-device from hidden_states).
  - Each core produces a partial [32, 4096] o_proj output; the "all-reduce"
    is a host-side sum of the 8 partials.

Per-core device kernel (everything fp16 in SBUF, fp32 accumulation in PSUM):
  phase 1: q/k projections as feature-major [512, 32] tiles (head-major),
           per-batch v_new projections token-major [4, 512].
  phase 2: per (b, h): scores^T [128kv x 4tok] tiles via K^T-as-weights
           matmuls (32 kv tiles + 1 new-token tile with causal mask),
           exp via ACT (softmax max-subtraction skipped: |scores| <~ 6),
           softmax sums via ones-vector matmuls + strided DVE reduce,
           unnormalized PV with V-as-weights (feature-major output),
           normalization via reciprocal + ones-broadcast matmul.
  phase 3: o_proj with attnout^T-as-weights -> partial [32, 4096] fp32.
"""

import numpy as np

import concourse.bacc as bacc
import concourse.mybir as mybir
import concourse.tile as tile
from concourse.bass_utils import run_bass_kernel_spmd

N_CORES = 8
B, S, H = 8, 4, 4096
NH = 32                 # total heads
HPC = NH // N_CORES     # heads per core = 4
HD = H // NH            # head dim = 128
POS = 4096              # cache positions attended (rows >= POS are overwritten)
NT = POS // 128         # kv tiles per (b, h) = 32
NTOK = B * S            # 32 query tokens, token index = 4*b + s
KPC = HPC * HD          # per-core feature slice = 512
SCALE = HD ** -0.5
NEG_INF = -1e9

F16 = mybir.dt.float16
F32 = mybir.dt.float32


def build_nc():
    nc = bacc.Bacc("TRN2", target_bir_lowering=False)

    xT = nc.dram_tensor("xT", [128, NT * NTOK], F16, kind="ExternalInput")
    wq = nc.dram_tensor("wq", [128, NT * KPC], F16, kind="ExternalInput")
    wk = nc.dram_tensor("wk", [128, NT * KPC], F16, kind="ExternalInput")
    wv = nc.dram_tensor("wv", [128, NT * KPC], F16, kind="ExternalInput")
    wo = nc.dram_tensor("wo", [128, HPC * H], F16, kind="ExternalInput")
    kt = nc.dram_tensor("kt", [B, 128, HPC * POS], F16, kind="ExternalInput")
    v = nc.dram_tensor("v", [B, 128, HPC * POS], F16, kind="ExternalInput")
    mask = nc.dram_tensor("mask", [S, S], F32, kind="ExternalInput")
    ones_col = nc.dram_tensor("ones_col", [128, 1], F16, kind="ExternalInput")
    ones_row = nc.dram_tensor("ones_row", [1, 128], F32, kind="ExternalInput")
    out = nc.dram_tensor("out", [NTOK, H], F32, kind="ExternalOutput")

    with tile.TileContext(nc) as tc:
        _body(tc, xT.ap(), wq.ap(), wk.ap(), wv.ap(), wo.ap(), kt.ap(), v.ap(),
              mask.ap(), ones_col.ap(), ones_row.ap(), out.ap())
    nc.compile()
    return nc


def _body(tc, xT, wq, wk, wv, wo, kt, v, mask, ones_col, ones_row, out):
    nc = tc.nc
    from contextlib import ExitStack
    ctx = ExitStack()
    with ctx:
        consts = ctx.enter_context(tc.tile_pool(name="consts", bufs=1))
        persist = ctx.enter_context(tc.tile_pool(name="persist", bufs=1))
        wpool = ctx.enter_context(tc.tile_pool(name="wpool", bufs=2))
        kvpool = ctx.enter_context(tc.tile_pool(name="kvpool", bufs=3))
        smpool = ctx.enter_context(tc.tile_pool(name="smpool", bufs=2))
        ps = ctx.enter_context(tc.tile_pool(name="ps", bufs=2, space="PSUM"))

        # ---- constants ----
        mask_sb = consts.tile([S, S], F32)
        nc.gpsimd.dma_start(out=mask_sb, in_=mask)
        onec_sb = consts.tile([128, 1], F16)
        nc.gpsimd.dma_start(out=onec_sb, in_=ones_col)
        oner_sb = consts.tile([1, 128], F32)
        nc.gpsimd.dma_start(out=oner_sb, in_=ones_row)

        xT_sb = persist.tile([128, NT * NTOK], F16)
        nc.gpsimd.dma_start(out=xT_sb, in_=xT)

        qT_sb = persist.tile([128, HPC * NTOK], F16)
        kT_sb = persist.tile([128, HPC * NTOK], F16)
        attnT_sb = persist.tile([128, HPC * NTOK], F16)
        vnew_sb = [persist.tile([S, KPC], F16, name=f"vnew{b}") for b in range(B)]

        # ---- phase 1: projections ----
        # q/k feature-major: per head m, [128 feat, 32 tok]
        for w_dram, dst in ((wq, qT_sb), (wk, kT_sb)):
            w_sb = wpool.tile([128, NT * KPC], F16, tag="w")
            nc.gpsimd.dma_start(out=w_sb, in_=w_dram)
            for m in range(HPC):
                pp = ps.tile([128, NTOK], F32, tag="big")
                for t in range(NT):
                    nc.tensor.matmul(
                        pp,
                        lhsT=w_sb[:, KPC * t + 128 * m: KPC * t + 128 * m + 128],
                        rhs=xT_sb[:, NTOK * t: NTOK * (t + 1)],
                        start=(t == 0), stop=(t == NT - 1),
                    )
                nc.scalar.copy(out=dst[:, NTOK * m: NTOK * (m + 1)], in_=pp)

        # v_new token-major per batch: [4 tok, 512 feat] at partitions 0..3
        wv_sb = wpool.tile([128, NT * KPC], F16, tag="w")
        nc.gpsimd.dma_start(out=wv_sb, in_=wv)
        for b in range(B):
            vp = ps.tile([S, KPC], F32, tag="big")
            for t in range(NT):
                nc.tensor.matmul(
                    vp,
                    lhsT=xT_sb[:, NTOK * t + S * b: NTOK * t + S * b + S],
                    rhs=wv_sb[:, KPC * t: KPC * (t + 1)],
                    start=(t == 0), stop=(t == NT - 1),
                )
            nc.scalar.copy(out=vnew_sb[b], in_=vp)

        # o_proj weights: traced here so the DMA overlaps the attention phase
        wo_sb = wpool.tile([128, HPC * H], F16, tag="w")
        nc.gpsimd.dma_start(out=wo_sb, in_=wo)

        # ---- phase 2: attention ----
        for b in range(B):
            for hp in range(HPC // 2):      # DMA granularity: 2 heads = 2MB
                ktch = kvpool.tile([128, 2 * POS], F16, tag="kt")
                nc.sync.dma_start(out=ktch, in_=kt[b][:, 2 * POS * hp: 2 * POS * (hp + 1)])
                vch = kvpool.tile([128, 2 * POS], F16, tag="v")
                nc.scalar.dma_start(out=vch, in_=v[b][:, 2 * POS * hp: 2 * POS * (hp + 1)])
                for hh in range(2):
                    h = 2 * hp + hh
                    col = NTOK * h + S * b  # (head, batch) column in qT/kT/attnT
                    scores = ps.tile([128, NT * S + S], F32, tag="big")
                    for t in range(NT):
                        nc.tensor.matmul(
                            scores[:, S * t: S * (t + 1)],
                            lhsT=ktch[:, POS * hh + 128 * t: POS * hh + 128 * t + 128],
                            rhs=qT_sb[:, col: col + S],
                            start=True, stop=True,
                        )
                    # new-token scores [4 kv_new, 4 tok] + causal mask
                    nc.tensor.matmul(
                        scores[0:S, NT * S: NT * S + S],
                        lhsT=kT_sb[:, col: col + S],
                        rhs=qT_sb[:, col: col + S],
                        start=True, stop=True,
                    )
                    nc.vector.tensor_add(
                        out=scores[0:S, NT * S: NT * S + S],
                        in0=scores[0:S, NT * S: NT * S + S],
                        in1=mask_sb,
                    )
                    probs = smpool.tile([128, NT * S + S], F16, tag="probs")
                    nc.scalar.activation(
                        out=probs[:, 0: NT * S], in_=scores[:, 0: NT * S],
                        func=mybir.ActivationFunctionType.Exp, scale=SCALE,
                    )
                    nc.scalar.activation(
                        out=probs[0:S, NT * S: NT * S + S],
                        in_=scores[0:S, NT * S: NT * S + S],
                        func=mybir.ActivationFunctionType.Exp, scale=SCALE,
                    )
                    # softmax denominators: [1, 33*4] partial sums
                    sums = ps.tile([1, NT * S + S], F32, tag="sums")
                    nc.tensor.matmul(
                        sums[0:1, 0: NT * S], lhsT=onec_sb, rhs=probs[:, 0: NT * S],
                        start=True, stop=True,
                    )
                    nc.tensor.matmul(
                        sums[0:1, NT * S: NT * S + S],
                        lhsT=onec_sb[0:S, 0:1],
                        rhs=probs[0:S, NT * S: NT * S + S],
                        start=True, stop=True,
                    )
                    total = smpool.tile([1, S], F32, tag="total")
                    nc.vector.reduce_sum(
                        out=total,
                        in_=sums[0:1, :].rearrange("p (t s) -> p s t", s=S),
                        axis=mybir.AxisListType.X,
                    )
                    rec = smpool.tile([1, S], F32, tag="rec")
                    nc.vector.reciprocal(out=rec, in_=total)
                    bcast = ps.tile([128, S], F32, tag="bcast")
                    nc.tensor.matmul(bcast, lhsT=oner_sb, rhs=rec, start=True, stop=True)
                    bcast_sb = smpool.tile([128, S], F32, tag="bcast_sb")
                    nc.scalar.copy(out=bcast_sb, in_=bcast)
                    # unnormalized PV, feature-major [128 feat, 4 tok]
                    outun = ps.tile([128, S], F32, tag="outun")
                    for t in range(NT):
                        nc.tensor.matmul(
                            outun,
                            lhsT=vch[:, POS * hh + 128 * t: POS * hh + 128 * t + 128],
                            rhs=probs[:, S * t: S * (t + 1)],
                            start=(t == 0), stop=False,
                        )
                    nc.tensor.matmul(
                        outun,
                        lhsT=vnew_sb[b][0:S, HD * h: HD * (h + 1)],
                        rhs=probs[0:S, NT * S: NT * S + S],
                        start=False, stop=True,
                    )
                    nc.vector.tensor_mul(
                        out=attnT_sb[:, col: col + S], in0=outun, in1=bcast_sb,
                    )

        # ---- phase 3: o_proj ----
        for n in range(H // 512):
            op = ps.tile([NTOK, 512], F32, tag="big")
            for j in range(HPC):
                nc.tensor.matmul(
                    op,
                    lhsT=attnT_sb[:, NTOK * j: NTOK * (j + 1)],
                    rhs=wo_sb[:, H * j + 512 * n: H * j + 512 * (n + 1)],
                    start=(j == 0), stop=(j == HPC - 1),
                )
            o_sb = smpool.tile([NTOK, 512], F32, tag="o_sb")
            nc.scalar.copy(out=o_sb, in_=op)
            nc.sync.dma_start(out=out[:, 512 * n: 512 * (n + 1)], in_=o_sb)


# ---------------------------------------------------------------------------
# host side
# ---------------------------------------------------------------------------

def build_core_inputs(hidden_states, Wq, Wk, Wv, Wo, key_cache, value_cache):
    """Shard + lay out the full inputs into the 8 per-core DRAM images."""
    tokens = np.ascontiguousarray(hidden_states.reshape(NTOK, H))
    xT = tokens.T.astype(np.float16)                       # [4096, 32]
    xT_sb = np.ascontiguousarray(
        xT.reshape(NT, 128, NTOK).transpose(1, 0, 2)).reshape(128, NT * NTOK)

    WqT = Wq.T.astype(np.float16)                          # [in=4096, out=4096]
    WkT = Wk.T.astype(np.float16)
    WvT = Wv.T.astype(np.float16)
    WoT = Wo.T.astype(np.float16)                          # [in, out]
    K16 = key_cache[:, :, :POS, :].astype(np.float16)      # [B, NH, POS, HD]
    V16 = value_cache[:, :, :POS, :].astype(np.float16)

    mask = np.where(np.arange(S)[:, None] > np.arange(S)[None, :],
                    np.float32(NEG_INF), np.float32(0.0))
    ones_col = np.ones((128, 1), np.float16)
    ones_row = np.ones((1, 128), np.float32)

    in_maps = []
    for c in range(N_CORES):
        cs = slice(KPC * c, KPC * (c + 1))
        hs = slice(HPC * c, HPC * (c + 1))

        def wlayout(WT):
            a = np.ascontiguousarray(WT[:, cs])            # [4096, 512]
            return np.ascontiguousarray(
                a.reshape(NT, 128, KPC).transpose(1, 0, 2)).reshape(128, NT * KPC)

        wo_c = np.ascontiguousarray(WoT[cs, :])            # [512, 4096]
        wo_c = np.ascontiguousarray(
            wo_c.reshape(HPC, 128, H).transpose(1, 0, 2)).reshape(128, HPC * H)

        kt_c = np.ascontiguousarray(
            K16[:, hs].transpose(0, 3, 1, 2)).reshape(B, 128, HPC * POS)
        v_c = np.ascontiguousarray(
            V16[:, hs].reshape(B, HPC, NT, 128, HD).transpose(0, 3, 1, 2, 4)
        ).reshape(B, 128, HPC * POS)

        in_maps.append({
            "xT": xT_sb, "wq": wlayout(WqT), "wk": wlayout(WkT),
            "wv": wlayout(WvT), "wo": wo_c, "kt": kt_c, "v": v_c,
            "mask": mask, "ones_col": ones_col, "ones_row": ones_row,
        })
    return in_maps


def numpy_core_kernel(m):
    """Numpy mirror of the device dataflow for one core (layout validation)."""
    f = np.float32
    xT_sb = m["xT"].astype(f)
    xT = xT_sb.reshape(128, NT, NTOK).transpose(1, 0, 2).reshape(H, NTOK)

    def unw(w):
        return w.astype(f).reshape(128, NT, KPC).transpose(1, 0, 2).reshape(H, KPC)

    qT = unw(m["wq"]).T @ xT            # [512 feat, 32 tok]
    kT = unw(m["wk"]).T @ xT
    vnew = (unw(m["wv"]).T @ xT).T      # [32 tok, 512 feat]
    qT = qT.astype(np.float16).astype(f)
    kT = kT.astype(np.float16).astype(f)
    vnew = vnew.astype(np.float16).astype(f)

    attnT = np.zeros((KPC, NTOK), f)
    for b in range(B):
        for h in range(HPC):
            colsl = slice(S * b, S * b + S)
            ktb = m["kt"][b].astype(f)   # [128, HPC*POS]
            vb = m["v"][b].astype(f)
            KTbh = ktb[:, POS * h: POS * (h + 1)]          # [hd, kv]
            scoresT = KTbh.T @ qT[HD * h: HD * (h + 1), colsl]   # [kv, 4]
            snew = kT[HD * h: HD * (h + 1), colsl].T @ qT[HD * h: HD * (h + 1), colsl]
            snew = snew + m["mask"]                        # [kv_new j, tok s]
            pr = np.exp(SCALE * scoresT).astype(np.float16).astype(f)
            prnew = np.exp(SCALE * snew).astype(np.float16).astype(f)
            den = pr.sum(axis=0) + prnew.sum(axis=0)
            V_bh = (vb[:, POS * h: POS * (h + 1)]
                    .reshape(128, NT, HD).transpose(1, 0, 2).reshape(POS, HD))
            ou = V_bh.T @ pr + vnew[S * b: S * b + S, KPC // 4 * 0 + HD * h: HD * (h + 1)].T @ prnew
            attnT[HD * h: HD * (h + 1), colsl] = (ou / den).astype(np.float16)
    woc = m["wo"].astype(f).reshape(128, HPC, H).transpose(1, 0, 2).reshape(KPC, H)
    return (attnT.astype(np.float16).astype(f).T @ woc).astype(np.float32)


_NC_CACHE = None


def get_nc():
    global _NC_CACHE
    if _NC_CACHE is None:
        _NC_CACHE = build_nc()
    return _NC_CACHE


def run_on_hw(inputs, trace=False, trace_cores=None):
    position = int(inputs["position"])
    assert position == POS, position
    in_maps = build_core_inputs(
        np.asarray(inputs["hidden_states"]), np.asarray(inputs["Wq"]),
        np.asarray(inputs["Wk"]), np.asarray(inputs["Wv"]), np.asarray(inputs["Wo"]),
        np.asarray(inputs["key_cache"]), np.asarray(inputs["value_cache"]))
    nc = get_nc()
    res = run_bass_kernel_spmd(nc, in_maps, core_ids=list(range(N_CORES)),
                               trace=trace, trace_cores=trace_cores)
    partial = np.zeros((NTOK, H), np.float64)
    for c in range(N_CORES):
        partial += res.results[c]["out"].astype(np.float64)
    out = partial.astype(np.float32).reshape(B, S, H)
    return out, res


def kernel(**inputs) -> np.ndarray:
    out, _ = run_on_hw(inputs, trace=False)
    return out


# revision 6
# speedup vs baseline: 1.0338x; 1.0338x over previous
"""Cached multi-head attention (decode-append, S=4) on 8 Trainium2 NeuronCores.

Sharding: tensor-parallel over the 32 heads -> 4 heads per core.
  - Wq/Wk/Wv split on the output-feature (head) axis, Wo on the input axis.
  - Each core holds its heads' slice of the KV cache (positions 0..4095; the
    4 new positions are computed on-device from hidden_states).
  - Each core produces a partial [32, 4096] o_proj output; the "all-reduce"
    is a host-side sum of the 8 partials.

Per-core device kernel (everything fp16 in SBUF, fp32 accumulation in PSUM):
  phase 1: q/k projections as feature-major [512, 32] tiles (head-major),
           per-batch v_new projections token-major [4, 512].
  phase 2: per (b, h): scores^T [128kv x 4tok] tiles via K^T-as-weights
           matmuls (32 kv tiles + 1 new-token tile with causal mask),
           exp via ACT (softmax max-subtraction skipped: |scores| <~ 6),
           softmax sums via ones-vector matmuls + strided DVE reduce,
           unnormalized PV with V-as-weights (feature-major output),
           normalization via reciprocal + ones-broadcast matmul.
  phase 3: o_proj with attnout^T-as-weights -> partial [32, 4096] fp32.
"""

import numpy as np

import concourse.bacc as bacc
import concourse.mybir as mybir
import concourse.tile as tile
from concourse.bass_utils import run_bass_kernel_spmd

N_CORES = 8
B, S, H = 8, 4, 4096
NH = 32                 # total heads
HPC = NH // N_CORES     # heads per core = 4
HD = H // NH            # head dim = 128
POS = 4096              # cache positions attended (rows >= POS are overwritten)
NT = POS // 128         # kv tiles per (b, h) = 32
NTOK = B * S            # 32 query tokens, token index = 4*b + s
KPC = HPC * HD          # per-core feature slice = 512
SCALE = HD ** -0.5
NEG_INF = -1e9

F16 = mybir.dt.float16
F32 = mybir.dt.float32


def build_nc():
    nc = bacc.Bacc("TRN2", target_bir_lowering=False)

    xT = nc.dram_tensor("xT", [128, NT * NTOK], F16, kind="ExternalInput")
    wq = nc.dram_tensor("wq", [128, NT * KPC], F16, kind="ExternalInput")
    wk = nc.dram_tensor("wk", [128, NT * KPC], F16, kind="ExternalInput")
    wv = nc.dram_tensor("wv", [128, NT * KPC], F16, kind="ExternalInput")
    wo = nc.dram_tensor("wo", [128, HPC * H], F16, kind="ExternalInput")
    kt = nc.dram_tensor("kt", [B, 128, HPC * POS], F16, kind="ExternalInput")
    v = nc.dram_tensor("v", [B, 128, HPC * POS], F16, kind="ExternalInput")
    mask = nc.dram_tensor("mask", [S, S], F32, kind="ExternalInput")
    ones_col = nc.dram_tensor("ones_col", [128, 1], F16, kind="ExternalInput")
    ones_row = nc.dram_tensor("ones_row", [1, 128], F32, kind="ExternalInput")
    out = nc.dram_tensor("out", [NTOK, H], F32, kind="ExternalOutput")

    with tile.TileContext(nc) as tc:
        _body(tc, xT.ap(), wq.ap(), wk.ap(), wv.ap(), wo.ap(), kt.ap(), v.ap(),
              mask.ap(), ones_col.ap(), ones_row.ap(), out.ap())
    nc.compile()
    return nc


def _body(tc, xT, wq, wk, wv, wo, kt, v, mask, ones_col, ones_row, out):
    nc = tc.nc
    from contextlib import ExitStack
    ctx = ExitStack()
    with ctx:
        consts = ctx.enter_context(tc.tile_pool(name="consts", bufs=1))
        persist = ctx.enter_context(tc.tile_pool(name="persist", bufs=1))
        wpool = ctx.enter_context(tc.tile_pool(name="wpool", bufs=2))
        kvpool = ctx.enter_context(tc.tile_pool(name="kvpool", bufs=3))
        smpool = ctx.enter_context(tc.tile_pool(name="smpool", bufs=2))
        ps = ctx.enter_context(tc.tile_pool(name="ps", bufs=2, space="PSUM"))

        # ---- constants ----
        mask_sb = consts.tile([S, S], F32)
        nc.gpsimd.dma_start(out=mask_sb, in_=mask)
        onec_sb = consts.tile([128, 1], F16)
        nc.gpsimd.dma_start(out=onec_sb, in_=ones_col)
        oner_sb = consts.tile([1, 128], F32)
        nc.gpsimd.dma_start(out=oner_sb, in_=ones_row)

        xT_sb = persist.tile([128, NT * NTOK], F16)
        nc.gpsimd.dma_start(out=xT_sb, in_=xT)

        qT_sb = persist.tile([128, HPC * NTOK], F16)
        kT_sb = persist.tile([128, HPC * NTOK], F16)
        attnT_sb = persist.tile([128, HPC * NTOK], F16)
        vnew_sb = [persist.tile([S, KPC], F16, name=f"vnew{b}") for b in range(B)]

        # ---- phase 1: projections ----
        # q/k feature-major: per head m, [128 feat, 32 tok]
        for w_dram, dst in ((wq, qT_sb), (wk, kT_sb)):
            w_sb = wpool.tile([128, NT * KPC], F16, tag="w")
            nc.gpsimd.dma_start(out=w_sb, in_=w_dram)
            for m in range(HPC):
                pp = ps.tile([128, NTOK], F32, tag="big")
                for t in range(NT):
                    nc.tensor.matmul(
                        pp,
                        lhsT=w_sb[:, KPC * t + 128 * m: KPC * t + 128 * m + 128],
                        rhs=xT_sb[:, NTOK * t: NTOK * (t + 1)],
                        start=(t == 0), stop=(t == NT - 1),
                    )
                nc.scalar.copy(out=dst[:, NTOK * m: NTOK * (m + 1)], in_=pp)

        # v_new token-major per batch: [4 tok, 512 feat] at partitions 0..3
        wv_sb = wpool.tile([128, NT * KPC], F16, tag="w")
        nc.gpsimd.dma_start(out=wv_sb, in_=wv)
        for b in range(B):
            vp = ps.tile([S, KPC], F32, tag="big")
            for t in range(NT):
                nc.tensor.matmul(
                    vp,
                    lhsT=xT_sb[:, NTOK * t + S * b: NTOK * t + S * b + S],
                    rhs=wv_sb[:, KPC * t: KPC * (t + 1)],
                    start=(t == 0), stop=(t == NT - 1),
                )
            nc.scalar.copy(out=vnew_sb[b], in_=vp)

        # o_proj weights: traced here so the DMA overlaps the attention phase
        wo_sb = wpool.tile([128, HPC * H], F16, tag="w")
        nc.gpsimd.dma_start(out=wo_sb, in_=wo)

        # ---- phase 2: attention ----
        for b in range(B):
            for hp in range(HPC // 2):      # DMA granularity: 2 heads = 2MB
                ktch = kvpool.tile([128, 2 * POS], F16, tag="kt")
                nc.sync.dma_start(out=ktch, in_=kt[b][:, 2 * POS * hp: 2 * POS * (hp + 1)])
                vch = kvpool.tile([128, 2 * POS], F16, tag="v")
                nc.sync.dma_start(out=vch, in_=v[b][:, 2 * POS * hp: 2 * POS * (hp + 1)])
                for hh in range(2):
                    h = 2 * hp + hh
                    col = NTOK * h + S * b  # (head, batch) column in qT/kT/attnT
                    scores = ps.tile([128, NT * S + S], F32, tag="big")
                    for t in range(NT):
                        nc.tensor.matmul(
                            scores[:, S * t: S * (t + 1)],
                            lhsT=ktch[:, POS * hh + 128 * t: POS * hh + 128 * t + 128],
                            rhs=qT_sb[:, col: col + S],
                            start=True, stop=True,
                        )
                    # new-token scores [4 kv_new, 4 tok] + causal mask
                    nc.tensor.matmul(
                        scores[0:S, NT * S: NT * S + S],
                        lhsT=kT_sb[:, col: col + S],
                        rhs=qT_sb[:, col: col + S],
                        start=True, stop=True,
                    )
                    nc.vector.tensor_add(
                        out=scores[0:S, NT * S: NT * S + S],
                        in0=scores[0:S, NT * S: NT * S + S],
                        in1=mask_sb,
                    )
                    probs = smpool.tile([128, NT * S + S], F16, tag="probs")
                    nc.scalar.activation(
                        out=probs[:, 0: NT * S], in_=scores[:, 0: NT * S],
                        func=mybir.ActivationFunctionType.Exp, scale=SCALE,
                    )
                    nc.scalar.activation(
                        out=probs[0:S, NT * S: NT * S + S],
                        in_=scores[0:S, NT * S: NT * S + S],
                        func=mybir.ActivationFunctionType.Exp, scale=SCALE,
                    )
                    # softmax denominators: [1, 33*4] partial sums
                    sums = ps.tile([1, NT * S + S], F32, tag="sums")
                    nc.tensor.matmul(
                        sums[0:1, 0: NT * S], lhsT=onec_sb, rhs=probs[:, 0: NT * S],
                        start=True, stop=True,
                    )
                    nc.tensor.matmul(
                        sums[0:1, NT * S: NT * S + S],
                        lhsT=onec_sb[0:S, 0:1],
                        rhs=probs[0:S, NT * S: NT * S + S],
                        start=True, stop=True,
                    )
                    total = smpool.tile([1, S], F32, tag="total")
                    nc.vector.reduce_sum(
                        out=total,
                        in_=sums[0:1, :].rearrange("p (t s) -> p s t", s=S),
                        axis=mybir.AxisListType.X,
                    )
                    rec = smpool.tile([1, S], F32, tag="rec")
                    nc.vector.reciprocal(out=rec, in_=total)
                    bcast = ps.tile([128, S], F32, tag="bcast")
                    nc.tensor.matmul(bcast, lhsT=oner_sb, rhs=rec, start=True, stop=True)
                    bcast_sb = smpool.tile([128, S], F32, tag="bcast_sb")
                    nc.scalar.copy(out=bcast_sb, in_=bcast)
                    # unnormalized PV, feature-major [128 feat, 4 tok]
                    outun = ps.tile([128, S], F32, tag="outun")
                    for t in range(NT):
                        nc.tensor.matmul(
                            outun,
                            lhsT=vch[:, POS * hh + 128 * t: POS * hh + 128 * t + 128],
                            rhs=probs[:, S * t: S * (t + 1)],
                            start=(t == 0), stop=False,
                        )
                    nc.tensor.matmul(
                        outun,
                        lhsT=vnew_sb[b][0:S, HD * h: HD * (h + 1)],
                        rhs=probs[0:S, NT * S: NT * S + S],
                        start=False, stop=True,
                    )
                    nc.vector.tensor_mul(
                        out=attnT_sb[:, col: col + S], in0=outun, in1=bcast_sb,
                    )

        # ---- phase 3: o_proj ----
        for n in range(H // 512):
            op = ps.tile([NTOK, 512], F32, tag="big")
            for j in range(HPC):
                nc.tensor.matmul(
                    op,
                    lhsT=attnT_sb[:, NTOK * j: NTOK * (j + 1)],
                    rhs=wo_sb[:, H * j + 512 * n: H * j + 512 * (n + 1)],
                    start=(j == 0), stop=(j == HPC - 1),
                )
            o_sb = smpool.tile([NTOK, 512], F32, tag="o_sb")
            nc.scalar.copy(out=o_sb, in_=op)
            nc.sync.dma_start(out=out[:, 512 * n: 512 * (n + 1)], in_=o_sb)


# ---------------------------------------------------------------------------
# host side
# ---------------------------------------------------------------------------

def build_core_inputs(hidden_states, Wq, Wk, Wv, Wo, key_cache, value_cache):
    """Shard + lay out the full inputs into the 8 per-core DRAM images."""
    tokens = np.ascontiguousarray(hidden_states.reshape(NTOK, H))
    xT = tokens.T.astype(np.float16)                       # [4096, 32]
    xT_sb = np.ascontiguousarray(
        xT.reshape(NT, 128, NTOK).transpose(1, 0, 2)).reshape(128, NT * NTOK)

    WqT = Wq.T.astype(np.float16)                          # [in=4096, out=4096]
    WkT = Wk.T.astype(np.float16)
    WvT = Wv.T.astype(np.float16)
    WoT = Wo.T.astype(np.float16)                          # [in, out]
    K16 = key_cache[:, :, :POS, :].astype(np.float16)      # [B, NH, POS, HD]
    V16 = value_cache[:, :, :POS, :].astype(np.float16)

    mask = np.where(np.arange(S)[:, None] > np.arange(S)[None, :],
                    np.float32(NEG_INF), np.float32(0.0))
    ones_col = np.ones((128, 1), np.float16)
    ones_row = np.ones((1, 128), np.float32)

    in_maps = []
    for c in range(N_CORES):
        cs = slice(KPC * c, KPC * (c + 1))
        hs = slice(HPC * c, HPC * (c + 1))

        def wlayout(WT):
            a = np.ascontiguousarray(WT[:, cs])            # [4096, 512]
            return np.ascontiguousarray(
                a.reshape(NT, 128, KPC).transpose(1, 0, 2)).reshape(128, NT * KPC)

        wo_c = np.ascontiguousarray(WoT[cs, :])            # [512, 4096]
        wo_c = np.ascontiguousarray(
            wo_c.reshape(HPC, 128, H).transpose(1, 0, 2)).reshape(128, HPC * H)

        kt_c = np.ascontiguousarray(
            K16[:, hs].transpose(0, 3, 1, 2)).reshape(B, 128, HPC * POS)
        v_c = np.ascontiguousarray(
            V16[:, hs].reshape(B, HPC, NT, 128, HD).transpose(0, 3, 1, 2, 4)
        ).reshape(B, 128, HPC * POS)

        in_maps.append({
            "xT": xT_sb, "wq": wlayout(WqT), "wk": wlayout(WkT),
            "wv": wlayout(WvT), "wo": wo_c, "kt": kt_c, "v": v_c,
            "mask": mask, "ones_col": ones_col, "ones_row": ones_row,
        })
    return in_maps


def numpy_core_kernel(m):
    """Numpy mirror of the device dataflow for one core (layout validation)."""
    f = np.float32
    xT_sb = m["xT"].astype(f)
    xT = xT_sb.reshape(128, NT, NTOK).transpose(1, 0, 2).reshape(H, NTOK)

    def unw(w):
        return w.astype(f).reshape(128, NT, KPC).transpose(1, 0, 2).reshape(H, KPC)

    qT = unw(m["wq"]).T @ xT            # [512 feat, 32 tok]
    kT = unw(m["wk"]).T @ xT
    vnew = (unw(m["wv"]).T @ xT).T      # [32 tok, 512 feat]
    qT = qT.astype(np.float16).astype(f)
    kT = kT.astype(np.float16).astype(f)
    vnew = vnew.astype(np.float16).astype(f)

    attnT = np.zeros((KPC, NTOK), f)
    for b in range(B):
        for h in range(HPC):
            colsl = slice(S * b, S * b + S)
            ktb = m["kt"][b].astype(f)   # [128, HPC*POS]
            vb = m["v"][b].astype(f)
            KTbh = ktb[:, POS * h: POS * (h + 1)]          # [hd, kv]
            scoresT = KTbh.T @ qT[HD * h: HD * (h + 1), colsl]   # [kv, 4]
            snew = kT[HD * h: HD * (h + 1), colsl].T @ qT[HD * h: HD * (h + 1), colsl]
            snew = snew + m["mask"]                        # [kv_new j, tok s]
            pr = np.exp(SCALE * scoresT).astype(np.float16).astype(f)
            prnew = np.exp(SCALE * snew).astype(np.float16).astype(f)
            den = pr.sum(axis=0) + prnew.sum(axis=0)
            V_bh = (vb[:, POS * h: POS * (h + 1)]
                    .reshape(128, NT, HD).transpose(1, 0, 2).reshape(POS, HD))
            ou = V_bh.T @ pr + vnew[S * b: S * b + S, KPC // 4 * 0 + HD * h: HD * (h + 1)].T @ prnew
            attnT[HD * h: HD * (h + 1), colsl] = (ou / den).astype(np.float16)
    woc = m["wo"].astype(f).reshape(128, HPC, H).transpose(1, 0, 2).reshape(KPC, H)
    return (attnT.astype(np.float16).astype(f).T @ woc).astype(np.float32)


_NC_CACHE = None


def get_nc():
    global _NC_CACHE
    if _NC_CACHE is None:
        _NC_CACHE = build_nc()
    return _NC_CACHE


def run_on_hw(inputs, trace=False, trace_cores=None):
    position = int(inputs["position"])
    assert position == POS, position
    in_maps = build_core_inputs(
        np.asarray(inputs["hidden_states"]), np.asarray(inputs["Wq"]),
        np.asarray(inputs["Wk"]), np.asarray(inputs["Wv"]), np.asarray(inputs["Wo"]),
        np.asarray(inputs["key_cache"]), np.asarray(inputs["value_cache"]))
    nc = get_nc()
    res = run_bass_kernel_spmd(nc, in_maps, core_ids=list(range(N_CORES)),
                               trace=trace, trace_cores=trace_cores)
    partial = np.zeros((NTOK, H), np.float64)
    for c in range(N_CORES):
        partial += res.results[c]["out"].astype(np.float64)
    out = partial.astype(np.float32).reshape(B, S, H)
    return out, res


def kernel(**inputs) -> np.ndarray:
    out, _ = run_on_hw(inputs, trace=False)
    return out


# revision 7
# speedup vs baseline: 1.0985x; 1.0626x over previous
"""Cached multi-head attention (decode-append, S=4) on 8 Trainium2 NeuronCores.

Sharding: tensor-parallel over the 32 heads -> 4 heads per core.
  - Wq/Wk/Wv split on the output-feature (head) axis, Wo on the input axis.
  - Each core holds its heads' slice of the KV cache (positions 0..4095; the
    4 new positions are computed on-device from hidden_states).
  - Each core produces a partial [32, 4096] o_proj output; the "all-reduce"
    is a host-side sum of the 8 partials.

Per-core device kernel (everything fp16 in SBUF, fp32 accumulation in PSUM):
  phase 1: q/k projections as feature-major [512, 32] tiles (head-major),
           per-batch v_new projections token-major [4, 512].
  phase 2: per (b, h): scores^T [128kv x 4tok] tiles via K^T-as-weights
           matmuls (32 kv tiles + 1 new-token tile with causal mask),
           exp via ACT (softmax max-subtraction skipped: |scores| <~ 6),
           softmax sums via ones-vector matmuls + strided DVE reduce,
           unnormalized PV with V-as-weights (feature-major output),
           normalization via reciprocal + ones-broadcast matmul.
  phase 3: o_proj with attnout^T-as-weights -> partial [32, 4096] fp32.
"""

import numpy as np

import concourse.bacc as bacc
import concourse.mybir as mybir
import concourse.tile as tile
from concourse.bass_utils import run_bass_kernel_spmd

N_CORES = 8
B, S, H = 8, 4, 4096
NH = 32                 # total heads
HPC = NH // N_CORES     # heads per core = 4
HD = H // NH            # head dim = 128
POS = 4096              # cache positions attended (rows >= POS are overwritten)
NT = POS // 128         # kv tiles per (b, h) = 32
NTOK = B * S            # 32 query tokens, token index = 4*b + s
KPC = HPC * HD          # per-core feature slice = 512
SCALE = HD ** -0.5
NEG_INF = -1e9

F16 = mybir.dt.float16
F32 = mybir.dt.float32


def build_nc():
    nc = bacc.Bacc("TRN2", target_bir_lowering=False)

    xT = nc.dram_tensor("xT", [128, NT * NTOK], F16, kind="ExternalInput")
    wq = nc.dram_tensor("wq", [128, NT * KPC], F16, kind="ExternalInput")
    wk = nc.dram_tensor("wk", [128, NT * KPC], F16, kind="ExternalInput")
    wv = nc.dram_tensor("wv", [128, NT * KPC], F16, kind="ExternalInput")
    wo = nc.dram_tensor("wo", [128, HPC * H], F16, kind="ExternalInput")
    kt = nc.dram_tensor("kt", [B, 128, HPC * POS], F16, kind="ExternalInput")
    v = nc.dram_tensor("v", [B, 128, HPC * POS], F16, kind="ExternalInput")
    mask = nc.dram_tensor("mask", [S, S], F32, kind="ExternalInput")
    ones_col = nc.dram_tensor("ones_col", [128, 1], F16, kind="ExternalInput")
    ones_row = nc.dram_tensor("ones_row", [1, 128], F32, kind="ExternalInput")
    out = nc.dram_tensor("out", [NTOK, H], F32, kind="ExternalOutput")

    with tile.TileContext(nc) as tc:
        _body(tc, xT.ap(), wq.ap(), wk.ap(), wv.ap(), wo.ap(), kt.ap(), v.ap(),
              mask.ap(), ones_col.ap(), ones_row.ap(), out.ap())
    nc.compile()
    return nc


def _body(tc, xT, wq, wk, wv, wo, kt, v, mask, ones_col, ones_row, out):
    nc = tc.nc
    from contextlib import ExitStack
    ctx = ExitStack()
    with ctx:
        consts = ctx.enter_context(tc.tile_pool(name="consts", bufs=1))
        persist = ctx.enter_context(tc.tile_pool(name="persist", bufs=1))
        wpool = ctx.enter_context(tc.tile_pool(name="wpool", bufs=2))
        kvpool = ctx.enter_context(tc.tile_pool(name="kvpool", bufs=2))
        smpool = ctx.enter_context(tc.tile_pool(name="smpool", bufs=2))
        ps = ctx.enter_context(tc.tile_pool(name="ps", bufs=2, space="PSUM"))

        # ---- constants ----
        mask_sb = consts.tile([S, S], F32)
        nc.sync.dma_start(out=mask_sb, in_=mask)
        onec_sb = consts.tile([128, 1], F16)
        nc.sync.dma_start(out=onec_sb, in_=ones_col)
        oner_sb = consts.tile([1, 128], F32)
        nc.sync.dma_start(out=oner_sb, in_=ones_row)

        xT_sb = persist.tile([128, NT * NTOK], F16)
        nc.sync.dma_start(out=xT_sb, in_=xT)

        qT_sb = persist.tile([128, HPC * NTOK], F16)
        kT_sb = persist.tile([128, HPC * NTOK], F16)
        attnT_sb = persist.tile([128, HPC * NTOK], F16)
        vnew_sb = [persist.tile([S, KPC], F16, name=f"vnew{b}") for b in range(B)]

        # ---- phase 1: projections ----
        # q/k feature-major: per head m, [128 feat, 32 tok]
        for w_dram, dst in ((wq, qT_sb), (wk, kT_sb)):
            w_sb = wpool.tile([128, NT * KPC], F16, tag="w")
            nc.sync.dma_start(out=w_sb, in_=w_dram)
            for m in range(HPC):
                pp = ps.tile([128, NTOK], F32, tag="big")
                for t in range(NT):
                    nc.tensor.matmul(
                        pp,
                        lhsT=w_sb[:, KPC * t + 128 * m: KPC * t + 128 * m + 128],
                        rhs=xT_sb[:, NTOK * t: NTOK * (t + 1)],
                        start=(t == 0), stop=(t == NT - 1),
                    )
                nc.scalar.copy(out=dst[:, NTOK * m: NTOK * (m + 1)], in_=pp)

        # v_new token-major per batch: [4 tok, 512 feat] at partitions 0..3
        wv_sb = wpool.tile([128, NT * KPC], F16, tag="w")
        nc.sync.dma_start(out=wv_sb, in_=wv)
        for b in range(B):
            vp = ps.tile([S, KPC], F32, tag="big")
            for t in range(NT):
                nc.tensor.matmul(
                    vp,
                    lhsT=xT_sb[:, NTOK * t + S * b: NTOK * t + S * b + S],
                    rhs=wv_sb[:, KPC * t: KPC * (t + 1)],
                    start=(t == 0), stop=(t == NT - 1),
                )
            nc.scalar.copy(out=vnew_sb[b], in_=vp)

        # o_proj weights: traced here so the DMA overlaps the attention phase
        wo_sb = wpool.tile([128, HPC * H], F16, tag="w")
        nc.sync.dma_start(out=wo_sb, in_=wo)

        # ---- phase 2: attention ----
        for b in range(B):
            for hp in range(HPC // 2):      # DMA granularity: 2 heads = 2MB
                ktch = kvpool.tile([128, 2 * POS], F16, tag="kt")
                nc.sync.dma_start(out=ktch, in_=kt[b][:, 2 * POS * hp: 2 * POS * (hp + 1)])
                vch = kvpool.tile([128, 2 * POS], F16, tag="v")
                nc.sync.dma_start(out=vch, in_=v[b][:, 2 * POS * hp: 2 * POS * (hp + 1)])
                for hh in range(2):
                    h = 2 * hp + hh
                    col = NTOK * h + S * b  # (head, batch) column in qT/kT/attnT
                    scores = ps.tile([128, NT * S + S], F32, tag="big")
                    for t in range(NT):
                        nc.tensor.matmul(
                            scores[:, S * t: S * (t + 1)],
                            lhsT=ktch[:, POS * hh + 128 * t: POS * hh + 128 * t + 128],
                            rhs=qT_sb[:, col: col + S],
                            start=True, stop=True,
                        )
                    # new-token scores [4 kv_new, 4 tok] + causal mask
                    nc.tensor.matmul(
                        scores[0:S, NT * S: NT * S + S],
                        lhsT=kT_sb[:, col: col + S],
                        rhs=qT_sb[:, col: col + S],
                        start=True, stop=True,
                    )
                    nc.vector.tensor_add(
                        out=scores[0:S, NT * S: NT * S + S],
                        in0=scores[0:S, NT * S: NT * S + S],
                        in1=mask_sb,
                    )
                    probs = smpool.tile([128, NT * S + S], F16, tag="probs")
                    nc.scalar.activation(
                        out=probs[:, 0: NT * S], in_=scores[:, 0: NT * S],
                        func=mybir.ActivationFunctionType.Exp, scale=SCALE,
                    )
                    nc.scalar.activation(
                        out=probs[0:S, NT * S: NT * S + S],
                        in_=scores[0:S, NT * S: NT * S + S],
                        func=mybir.ActivationFunctionType.Exp, scale=SCALE,
                    )
                    # softmax denominators: [1, 33*4] partial sums
                    sums = ps.tile([1, NT * S + S], F32, tag="sums")
                    nc.tensor.matmul(
                        sums[0:1, 0: NT * S], lhsT=onec_sb, rhs=probs[:, 0: NT * S],
                        start=True, stop=True,
                    )
                    nc.tensor.matmul(
                        sums[0:1, NT * S: NT * S + S],
                        lhsT=onec_sb[0:S, 0:1],
                        rhs=probs[0:S, NT * S: NT * S + S],
                        start=True, stop=True,
                    )
                    total = smpool.tile([1, S], F32, tag="total")
                    nc.vector.reduce_sum(
                        out=total,
                        in_=sums[0:1, :].rearrange("p (t s) -> p s t", s=S),
                        axis=mybir.AxisListType.X,
                    )
                    rec = smpool.tile([1, S], F32, tag="rec")
                    nc.vector.reciprocal(out=rec, in_=total)
                    bcast = ps.tile([128, S], F32, tag="bcast")
                    nc.tensor.matmul(bcast, lhsT=oner_sb, rhs=rec, start=True, stop=True)
                    bcast_sb = smpool.tile([128, S], F32, tag="bcast_sb")
                    nc.scalar.copy(out=bcast_sb, in_=bcast)
                    # unnormalized PV, feature-major [128 feat, 4 tok]
                    outun = ps.tile([128, S], F32, tag="outun")
                    for t in range(NT):
                        nc.tensor.matmul(
                            outun,
                            lhsT=vch[:, POS * hh + 128 * t: POS * hh + 128 * t + 128],
                            rhs=probs[:, S * t: S * (t + 1)],
                            start=(t == 0), stop=False,
                        )
                    nc.tensor.matmul(
                        outun,
                        lhsT=vnew_sb[b][0:S, HD * h: HD * (h + 1)],
                        rhs=probs[0:S, NT * S: NT * S + S],
                        start=False, stop=True,
                    )
                    nc.vector.tensor_mul(
                        out=attnT_sb[:, col: col + S], in0=outun, in1=bcast_sb,
                    )

        # ---- phase 3: o_proj ----
        for n in range(H // 512):
            op = ps.tile([NTOK, 512], F32, tag="big")
            for j in range(HPC):
                nc.tensor.matmul(
                    op,
                    lhsT=attnT_sb[:, NTOK * j: NTOK * (j + 1)],
                    rhs=wo_sb[:, H * j + 512 * n: H * j + 512 * (n + 1)],
                    start=(j == 0), stop=(j == HPC - 1),
                )
            o_sb = smpool.tile([NTOK, 512], F32, tag="o_sb")
            nc.scalar.copy(out=o_sb, in_=op)
            nc.sync.dma_start(out=out[:, 512 * n: 512 * (n + 1)], in_=o_sb)


# ---------------------------------------------------------------------------
# host side
# ---------------------------------------------------------------------------

def build_core_inputs(hidden_states, Wq, Wk, Wv, Wo, key_cache, value_cache):
    """Shard + lay out the full inputs into the 8 per-core DRAM images."""
    tokens = np.ascontiguousarray(hidden_states.reshape(NTOK, H))
    xT = tokens.T.astype(np.float16)                       # [4096, 32]
    xT_sb = np.ascontiguousarray(
        xT.reshape(NT, 128, NTOK).transpose(1, 0, 2)).reshape(128, NT * NTOK)

    WqT = Wq.T.astype(np.float16)                          # [in=4096, out=4096]
    WkT = Wk.T.astype(np.float16)
    WvT = Wv.T.astype(np.float16)
    WoT = Wo.T.astype(np.float16)                          # [in, out]
    K16 = key_cache[:, :, :POS, :].astype(np.float16)      # [B, NH, POS, HD]
    V16 = value_cache[:, :, :POS, :].astype(np.float16)

    mask = np.where(np.arange(S)[:, None] > np.arange(S)[None, :],
                    np.float32(NEG_INF), np.float32(0.0))
    ones_col = np.ones((128, 1), np.float16)
    ones_row = np.ones((1, 128), np.float32)

    in_maps = []
    for c in range(N_CORES):
        cs = slice(KPC * c, KPC * (c + 1))
        hs = slice(HPC * c, HPC * (c + 1))

        def wlayout(WT):
            a = np.ascontiguousarray(WT[:, cs])            # [4096, 512]
            return np.ascontiguousarray(
                a.reshape(NT, 128, KPC).transpose(1, 0, 2)).reshape(128, NT * KPC)

        wo_c = np.ascontiguousarray(WoT[cs, :])            # [512, 4096]
        wo_c = np.ascontiguousarray(
            wo_c.reshape(HPC, 128, H).transpose(1, 0, 2)).reshape(128, HPC * H)

        kt_c = np.ascontiguousarray(
            K16[:, hs].transpose(0, 3, 1, 2)).reshape(B, 128, HPC * POS)
        v_c = np.ascontiguousarray(
            V16[:, hs].reshape(B, HPC, NT, 128, HD).transpose(0, 3, 1, 2, 4)
        ).reshape(B, 128, HPC * POS)

        in_maps.append({
            "xT": xT_sb, "wq": wlayout(WqT), "wk": wlayout(WkT),
            "wv": wlayout(WvT), "wo": wo_c, "kt": kt_c, "v": v_c,
            "mask": mask, "ones_col": ones_col, "ones_row": ones_row,
        })
    return in_maps


def numpy_core_kernel(m):
    """Numpy mirror of the device dataflow for one core (layout validation)."""
    f = np.float32
    xT_sb = m["xT"].astype(f)
    xT = xT_sb.reshape(128, NT, NTOK).transpose(1, 0, 2).reshape(H, NTOK)

    def unw(w):
        return w.astype(f).reshape(128, NT, KPC).transpose(1, 0, 2).reshape(H, KPC)

    qT = unw(m["wq"]).T @ xT            # [512 feat, 32 tok]
    kT = unw(m["wk"]).T @ xT
    vnew = (unw(m["wv"]).T @ xT).T      # [32 tok, 512 feat]
    qT = qT.astype(np.float16).astype(f)
    kT = kT.astype(np.float16).astype(f)
    vnew = vnew.astype(np.float16).astype(f)

    attnT = np.zeros((KPC, NTOK), f)
    for b in range(B):
        for h in range(HPC):
            colsl = slice(S * b, S * b + S)
            ktb = m["kt"][b].astype(f)   # [128, HPC*POS]
            vb = m["v"][b].astype(f)
            KTbh = ktb[:, POS * h: POS * (h + 1)]          # [hd, kv]
            scoresT = KTbh.T @ qT[HD * h: HD * (h + 1), colsl]   # [kv, 4]
            snew = kT[HD * h: HD * (h + 1), colsl].T @ qT[HD * h: HD * (h + 1), colsl]
            snew = snew + m["mask"]                        # [kv_new j, tok s]
            pr = np.exp(SCALE * scoresT).astype(np.float16).astype(f)
            prnew = np.exp(SCALE * snew).astype(np.float16).astype(f)
            den = pr.sum(axis=0) + prnew.sum(axis=0)
            V_bh = (vb[:, POS * h: POS * (h + 1)]
                    .reshape(128, NT, HD).transpose(1, 0, 2).reshape(POS, HD))
            ou = V_bh.T @ pr + vnew[S * b: S * b + S, KPC // 4 * 0 + HD * h: HD * (h + 1)].T @ prnew
            attnT[HD * h: HD * (h + 1), colsl] = (ou / den).astype(np.float16)
    woc = m["wo"].astype(f).reshape(128, HPC, H).transpose(1, 0, 2).reshape(KPC, H)
    return (attnT.astype(np.float16).astype(f).T @ woc).astype(np.float32)


_NC_CACHE = None


def get_nc():
    global _NC_CACHE
    if _NC_CACHE is None:
        _NC_CACHE = build_nc()
    return _NC_CACHE


def run_on_hw(inputs, trace=False, trace_cores=None):
    position = int(inputs["position"])
    assert position == POS, position
    in_maps = build_core_inputs(
        np.asarray(inputs["hidden_states"]), np.asarray(inputs["Wq"]),
        np.asarray(inputs["Wk"]), np.asarray(inputs["Wv"]), np.asarray(inputs["Wo"]),
        np.asarray(inputs["key_cache"]), np.asarray(inputs["value_cache"]))
    nc = get_nc()
    res = run_bass_kernel_spmd(nc, in_maps, core_ids=list(range(N_CORES)),
                               trace=trace, trace_cores=trace_cores)
    partial = np.zeros((NTOK, H), np.float64)
    for c in range(N_CORES):
        partial += res.results[c]["out"].astype(np.float64)
    out = partial.astype(np.float32).reshape(B, S, H)
    return out, res


def kernel(**inputs) -> np.ndarray:
    out, _ = run_on_hw(inputs, trace=False)
    return out


# revision 9
# speedup vs baseline: 1.2709x; 1.1569x over previous
"""Cached multi-head attention (decode-append, S=4) on 8 Trainium2 NeuronCores.

Sharding: tensor-parallel over the 32 heads -> 4 heads per core.
  - Wq/Wk/Wv split on the output-feature (head) axis, Wo on the input axis.
  - Each core holds its heads' slice of the KV cache (positions 0..4095; the
    4 new positions are computed on-device from hidden_states).
  - Each core produces a partial [32, 4096] o_proj output; the "all-reduce"
    is a host-side sum of the 8 partials.

Per-core device kernel (fp16 streams, fp32 accumulation in PSUM):
  phase 1: x-stationary projections -> q/k/v token-major [32, 512], then PE
           transposes for feature-major qT/kT; per-batch v_new slices (with a
           ones column) via SBUF->SBUF DMA.
  phase 2: per (b, h): scores^T [128kv x 4tok] tiles via K^T-as-weights
           matmuls (32 kv tiles + 1 new-token tile with causal mask),
           exp via ACT (softmax max-subtraction skipped: |scores| <~ 6),
           PV with probsT-as-weights streaming V|ones [128, 129] -> the
           ones column accumulates the softmax denominator for free,
           normalize via reciprocal + per-token scalar mul, PE transpose
           to feature-major attnT.
  phase 3: o_proj with attnT-as-weights -> partial [32, 4096] fp32.
"""

import numpy as np

import concourse.bacc as bacc
import concourse.mybir as mybir
import concourse.tile as tile
from concourse.bass_utils import run_bass_kernel_spmd

N_CORES = 8
B, S, H = 8, 4, 4096
NH = 32                 # total heads
HPC = NH // N_CORES     # heads per core = 4
HD = H // NH            # head dim = 128
POS = 4096              # cache positions attended (rows >= POS are overwritten)
NT = POS // 128         # kv tiles per (b, h) = 32
NTOK = B * S            # 32 query tokens, token index = 4*b + s
KPC = HPC * HD          # per-core feature slice = 512
VW = HD + 1             # v tile width with ones column = 129
SCALE = HD ** -0.5
NEG_INF = -1e9

F16 = mybir.dt.float16
F32 = mybir.dt.float32


def build_nc():
    nc = bacc.Bacc("TRN2", target_bir_lowering=False)

    xT = nc.dram_tensor("xT", [128, NT * NTOK], F16, kind="ExternalInput")
    wq = nc.dram_tensor("wq", [128, NT * KPC], F16, kind="ExternalInput")
    wk = nc.dram_tensor("wk", [128, NT * KPC], F16, kind="ExternalInput")
    wv = nc.dram_tensor("wv", [128, NT * KPC], F16, kind="ExternalInput")
    wo = nc.dram_tensor("wo", [128, HPC * H], F16, kind="ExternalInput")
    kt = nc.dram_tensor("kt", [B, 128, HPC * POS], F16, kind="ExternalInput")
    v = nc.dram_tensor("v", [B, 128, HPC * NT * VW], F16, kind="ExternalInput")
    mask = nc.dram_tensor("mask", [S, S], F32, kind="ExternalInput")
    ident = nc.dram_tensor("ident", [32, 32], F16, kind="ExternalInput")
    out = nc.dram_tensor("out", [NTOK, H], F32, kind="ExternalOutput")

    with tile.TileContext(nc) as tc:
        _body(tc, xT.ap(), wq.ap(), wk.ap(), wv.ap(), wo.ap(), kt.ap(), v.ap(),
              mask.ap(), ident.ap(), out.ap())
    nc.compile()
    return nc


def _body(tc, xT, wq, wk, wv, wo, kt, v, mask, ident, out):
    nc = tc.nc
    from contextlib import ExitStack
    Exp = mybir.ActivationFunctionType.Exp
    ctx = ExitStack()
    with ctx:
        consts = ctx.enter_context(tc.tile_pool(name="consts", bufs=1))
        persist = ctx.enter_context(tc.tile_pool(name="persist", bufs=1))
        wpool = ctx.enter_context(tc.tile_pool(name="wpool", bufs=3))
        kvpool = ctx.enter_context(tc.tile_pool(name="kvpool", bufs=2))
        smpool = ctx.enter_context(tc.tile_pool(name="smpool", bufs=2))
        ps = ctx.enter_context(tc.tile_pool(name="ps", bufs=2, space="PSUM"))

        # ---- constants ----
        mask_sb = consts.tile([S, S], F32)
        nc.sync.dma_start(out=mask_sb, in_=mask)
        id_sb = consts.tile([32, 32], F16)
        nc.sync.dma_start(out=id_sb, in_=ident)

        xT_sb = persist.tile([128, NT * NTOK], F16)
        nc.sync.dma_start(out=xT_sb, in_=xT)

        qT_sb = persist.tile([128, HPC * NTOK], F16)
        kT_sb = persist.tile([128, HPC * NTOK], F16)
        attnT_sb = persist.tile([128, HPC * NTOK], F16)
        vnew_sb = [persist.tile([S, HPC * VW], F16, name=f"vnew{b}") for b in range(B)]

        # ---- phase 1: projections (x-stationary, token-major) ----
        wq_sb = wpool.tile([128, NT * KPC], F16, tag="w")
        nc.sync.dma_start(out=wq_sb, in_=wq)
        wk_sb = wpool.tile([128, NT * KPC], F16, tag="w")
        nc.sync.dma_start(out=wk_sb, in_=wk)
        wv_sb = wpool.tile([128, NT * KPC], F16, tag="w")
        nc.sync.dma_start(out=wv_sb, in_=wv)

        q_ps = ps.tile([NTOK, KPC], F32, tag="scores")
        k_ps = ps.tile([NTOK, KPC], F32, tag="out4")
        v_ps = ps.tile([NTOK, KPC], F32, tag="tpose")
        for t in range(NT):
            lx = xT_sb[:, NTOK * t: NTOK * (t + 1)]
            st, sp = (t == 0), (t == NT - 1)
            nc.tensor.matmul(q_ps, lhsT=lx, rhs=wq_sb[:, KPC * t: KPC * (t + 1)], start=st, stop=sp)
            nc.tensor.matmul(k_ps, lhsT=lx, rhs=wk_sb[:, KPC * t: KPC * (t + 1)], start=st, stop=sp)
            nc.tensor.matmul(v_ps, lhsT=lx, rhs=wv_sb[:, KPC * t: KPC * (t + 1)], start=st, stop=sp)

        q_tok = persist.tile([NTOK, KPC], F16)
        k_tok = persist.tile([NTOK, KPC], F16)
        v_tok = persist.tile([NTOK, KPC], F16)
        nc.scalar.copy(out=q_tok, in_=q_ps)
        nc.scalar.copy(out=k_tok, in_=k_ps)
        nc.scalar.copy(out=v_tok, in_=v_ps)

        # feature-major qT/kT via PE transpose of [32, 128] chunks
        for src, dst in ((q_tok, qT_sb), (k_tok, kT_sb)):
            for m in range(HPC):
                tp = ps.tile([128, NTOK], F16, tag="tpose")
                nc.tensor.transpose(tp, in_=src[:, HD * m: HD * (m + 1)], identity=id_sb)
                nc.scalar.copy(out=dst[:, NTOK * m: NTOK * (m + 1)], in_=tp)

        # per-batch v_new [4, 4*129] (ones col per head) at partitions 0..3
        for b in range(B):
            vb = vnew_sb[b].rearrange("p (h d) -> p h d", d=VW)
            nc.vector.memset(vb[:, :, HD:VW], 1.0)
            nc.gpsimd.dma_start(
                out=vb[:, :, 0:HD],
                in_=v_tok[S * b: S * (b + 1), :].rearrange("p (h d) -> p h d", d=HD),
            )

        # o_proj weights: traced here so the DMA overlaps the attention phase
        wo_sb = wpool.tile([128, HPC * H], F16, tag="w")
        nc.sync.dma_start(out=wo_sb, in_=wo)

        # ---- phase 2: attention ----
        for b in range(B):
            for hp in range(HPC // 2):      # DMA granularity: 2 heads
                ktch = kvpool.tile([128, 2 * POS], F16, tag="kt")
                nc.sync.dma_start(out=ktch, in_=kt[b][:, 2 * POS * hp: 2 * POS * (hp + 1)])
                vch = kvpool.tile([128, 2 * NT * VW], F16, tag="v")
                nc.sync.dma_start(out=vch, in_=v[b][:, 2 * NT * VW * hp: 2 * NT * VW * (hp + 1)])
                for hh in range(2):
                    h = 2 * hp + hh
                    col = NTOK * h + S * b  # (head, batch) column in qT/kT/attnT
                    scores = ps.tile([128, NT * S + S], F32, tag="scores")
                    for t in range(NT):
                        nc.tensor.matmul(
                            scores[:, S * t: S * (t + 1)],
                            lhsT=ktch[:, POS * hh + 128 * t: POS * hh + 128 * t + 128],
                            rhs=qT_sb[:, col: col + S],
                            start=True, stop=True,
                        )
                    # new-token scores [4 kv_new, 4 tok] + causal mask
                    nc.tensor.matmul(
                        scores[0:S, NT * S: NT * S + S],
                        lhsT=kT_sb[:, col: col + S],
                        rhs=qT_sb[:, col: col + S],
                        start=True, stop=True,
                    )
                    nc.vector.tensor_add(
                        out=scores[0:S, NT * S: NT * S + S],
                        in0=scores[0:S, NT * S: NT * S + S],
                        in1=mask_sb,
                    )
                    probs = smpool.tile([128, NT * S + S], F16, tag="probs")
                    nc.scalar.activation(out=probs[:, 0: NT * S], in_=scores[:, 0: NT * S],
                                         func=Exp, scale=SCALE)
                    nc.scalar.activation(out=probs[0:S, NT * S: NT * S + S],
                                         in_=scores[0:S, NT * S: NT * S + S],
                                         func=Exp, scale=SCALE)
                    # PV: probsT stationary, V|ones streaming; col 128 = denom
                    out4 = ps.tile([S, VW], F32, tag="out4")
                    for t in range(NT):
                        nc.tensor.matmul(
                            out4,
                            lhsT=probs[:, S * t: S * (t + 1)],
                            rhs=vch[:, NT * VW * hh + VW * t: NT * VW * hh + VW * (t + 1)],
                            start=(t == 0), stop=False,
                        )
                    nc.tensor.matmul(
                        out4,
                        lhsT=probs[0:S, NT * S: NT * S + S],
                        rhs=vnew_sb[b][:, VW * h: VW * (h + 1)],
                        start=False, stop=True,
                    )
                    rec = smpool.tile([S, 1], F32, tag="rec")
                    nc.vector.reciprocal(out=rec, in_=out4[0:S, HD:VW])
                    atok = smpool.tile([S, HD], F16, tag="atok")
                    nc.vector.tensor_scalar_mul(atok, in0=out4[0:S, 0:HD], scalar1=rec)
                    tp = ps.tile([128, S], F16, tag="tpose")
                    nc.tensor.transpose(tp, in_=atok, identity=id_sb[0:S, 0:S])
                    nc.scalar.copy(out=attnT_sb[:, col: col + S], in_=tp)

        # ---- phase 3: o_proj ----
        for n in range(H // 512):
            op = ps.tile([NTOK, 512], F32, tag="scores")
            for j in range(HPC):
                nc.tensor.matmul(
                    op,
                    lhsT=attnT_sb[:, NTOK * j: NTOK * (j + 1)],
                    rhs=wo_sb[:, H * j + 512 * n: H * j + 512 * (n + 1)],
                    start=(j == 0), stop=(j == HPC - 1),
                )
            o_sb = smpool.tile([NTOK, 512], F32, tag="o_sb")
            nc.scalar.copy(out=o_sb, in_=op)
            nc.sync.dma_start(out=out[:, 512 * n: 512 * (n + 1)], in_=o_sb)


# ---------------------------------------------------------------------------
# host side
# ---------------------------------------------------------------------------

def build_core_inputs(hidden_states, Wq, Wk, Wv, Wo, key_cache, value_cache):
    """Shard + lay out the full inputs into the 8 per-core DRAM images."""
    tokens = np.ascontiguousarray(hidden_states.reshape(NTOK, H))
    xT = tokens.T.astype(np.float16)                       # [4096, 32]
    xT_sb = np.ascontiguousarray(
        xT.reshape(NT, 128, NTOK).transpose(1, 0, 2)).reshape(128, NT * NTOK)

    WqT = Wq.T.astype(np.float16)                          # [in=4096, out=4096]
    WkT = Wk.T.astype(np.float16)
    WvT = Wv.T.astype(np.float16)
    WoT = Wo.T.astype(np.float16)                          # [in, out]
    K16 = key_cache[:, :, :POS, :].astype(np.float16)      # [B, NH, POS, HD]
    V16 = value_cache[:, :, :POS, :].astype(np.float16)

    mask = np.where(np.arange(S)[:, None] > np.arange(S)[None, :],
                    np.float32(NEG_INF), np.float32(0.0))
    ident = np.eye(32, dtype=np.float16)

    in_maps = []
    for c in range(N_CORES):
        cs = slice(KPC * c, KPC * (c + 1))
        hs = slice(HPC * c, HPC * (c + 1))

        def wlayout(WT):
            a = np.ascontiguousarray(WT[:, cs])            # [4096, 512]
            return np.ascontiguousarray(
                a.reshape(NT, 128, KPC).transpose(1, 0, 2)).reshape(128, NT * KPC)

        wo_c = np.ascontiguousarray(WoT[cs, :])            # [512, 4096]
        wo_c = np.ascontiguousarray(
            wo_c.reshape(HPC, 128, H).transpose(1, 0, 2)).reshape(128, HPC * H)

        kt_c = np.ascontiguousarray(
            K16[:, hs].transpose(0, 3, 1, 2)).reshape(B, 128, HPC * POS)
        v_p = V16[:, hs].reshape(B, HPC, NT, 128, HD)      # [b, h, t, kv, d]
        v_aug = np.ones((B, HPC, NT, 128, VW), np.float16)
        v_aug[..., :HD] = v_p
        v_c = np.ascontiguousarray(
            v_aug.transpose(0, 3, 1, 2, 4)).reshape(B, 128, HPC * NT * VW)

        in_maps.append({
            "xT": xT_sb, "wq": wlayout(WqT), "wk": wlayout(WkT),
            "wv": wlayout(WvT), "wo": wo_c, "kt": kt_c, "v": v_c,
            "mask": mask, "ident": ident,
        })
    return in_maps


def numpy_core_kernel(m):
    """Numpy mirror of the device dataflow for one core (layout validation)."""
    f = np.float32
    f16 = np.float16
    xT_sb = m["xT"].astype(f)
    xT = xT_sb.reshape(128, NT, NTOK).transpose(1, 0, 2).reshape(H, NTOK)

    def unw(w):
        return w.astype(f).reshape(128, NT, KPC).transpose(1, 0, 2).reshape(H, KPC)

    qT = (unw(m["wq"]).T @ xT).astype(f16).astype(f)      # [512 feat, 32 tok]
    kT = (unw(m["wk"]).T @ xT).astype(f16).astype(f)
    vnew = (unw(m["wv"]).T @ xT).T.astype(f16).astype(f)  # [32 tok, 512 feat]

    attnT = np.zeros((KPC, NTOK), f)
    for b in range(B):
        for h in range(HPC):
            colsl = slice(S * b, S * b + S)
            KTbh = m["kt"][b].astype(f)[:, POS * h: POS * (h + 1)]   # [hd, kv]
            scoresT = KTbh.T @ qT[HD * h: HD * (h + 1), colsl]       # [kv, 4]
            snew = kT[HD * h: HD * (h + 1), colsl].T @ qT[HD * h: HD * (h + 1), colsl]
            snew = snew + m["mask"]                                  # [j, s]
            pr = np.exp(SCALE * scoresT).astype(f16).astype(f)
            prnew = np.exp(SCALE * snew).astype(f16).astype(f)
            den = pr.sum(axis=0) + prnew.sum(axis=0)
            vb = m["v"][b].astype(f)[:, NT * VW * h: NT * VW * (h + 1)]
            V_bh = vb.reshape(128, NT, VW)[:, :, :HD].transpose(1, 0, 2).reshape(POS, HD)
            ou = V_bh.T @ pr + vnew[S * b: S * b + S, HD * h: HD * (h + 1)].T @ prnew
            attnT[HD * h: HD * (h + 1), colsl] = (ou / den).astype(f16)
    woc = m["wo"].astype(f).reshape(128, HPC, H).transpose(1, 0, 2).reshape(KPC, H)
    return (attnT.astype(f16).astype(f).T @ woc).astype(np.float32)


_NC_CACHE = None


def get_nc():
    global _NC_CACHE
    if _NC_CACHE is None:
        _NC_CACHE = build_nc()
    return _NC_CACHE


def run_on_hw(inputs, trace=False, trace_cores=None):
    position = int(inputs["position"])
    assert position == POS, position
    in_maps = build_core_inputs(
        np.asarray(inputs["hidden_states"]), np.asarray(inputs["Wq"]),
        np.asarray(inputs["Wk"]), np.asarray(inputs["Wv"]), np.asarray(inputs["Wo"]),
        np.asarray(inputs["key_cache"]), np.asarray(inputs["value_cache"]))
    nc = get_nc()
    res = run_bass_kernel_spmd(nc, in_maps, core_ids=list(range(N_CORES)),
                               trace=trace, trace_cores=trace_cores)
    partial = np.zeros((NTOK, H), np.float64)
    for c in range(N_CORES):
        partial += res.results[c]["out"].astype(np.float64)
    out = partial.astype(np.float32).reshape(B, S, H)
    return out, res


def kernel(**inputs) -> np.ndarray:
    out, _ = run_on_hw(inputs, trace=False)
    return out
